# revision 20
# baseline (speedup 1.0000x reference)
"""Cross-view attention (nn_CrossViewAttention) Trainium2 Bass kernel.

Reference computation (B=2, N=4096, D=512):
    co    = relu(concat([x_f, x_s], -1) @ Wc.T + bc)
    out_f = attend(x_f@Wq.T+bq, x_s@Wk.T+bk, x_f@Wv.T+bv) + co
    out_s = attend(x_s@Wq.T+bq, x_f@Wk.T+bk, x_s@Wv.T+bv) + co
    attend(Q,K,V) = (softmax(Q K^T) / L1 / sqrt(D)) @ V

Sharding: 8 cores = (direction f/s) x (batch 0/1) x (sequence half).
Each core computes 2048 output rows of one direction against the full
4096-row K/V for its (direction, batch), SPMD with per-core input data.
Rows are permuted host-side so every core's own rows come first; the
attention reduction over keys is permutation invariant.  Collectives
proved to serialize the whole kernel for a minor PE saving, so K/V/co
are computed fully locally.

Design notes (v3):
  - Q/K/V projections run in fp8-e4m3 DoubleRow (host ships e4m3 copies
    of x^T and the weights); the co MLP stays bf16 (co dominates the
    output magnitude; fp8 there fails the accuracy gate).  bf16 x ships
    only for the core's own 2048 rows (co's input).
  - Every DMA-able tensor is pre-arranged on the host so each partition
    reads one fat contiguous run (4-16 KiB descriptors); the v2 layout
    produced 0.5-1 KiB descriptors that starved the first matmuls.
  - Input DMAs split across the two HWDGE queues (sync carries the K/Q
    operands, scalar everything else) so the first matmuls start right
    after the NEFF preamble.
  - Projections accumulate into [128,2048] PSUM tiles (4 banks, one
    8-bank pool of 2) and drain with FD=2048 ACT ops, so ACT no longer
    paces the PE during phase 1 (v2 lost ~146ns per 2-matmul group).
  - co and PV produce TRANSPOSED tiles (partitions = output feature):
    bc+relu fuse into the co ACT drain as a per-partition bias, bv
    enters the PV PSUM group via a ones-row matmul (rows sum to 1 after
    L1-normalize, so +bv there lands as +bv/sqrt(D) in the output), and
    the epilogue stays one fused DVE op out^T = pv^T/sqrt(D) + co^T.
    The kernel emits out^T [D, nq]; the host transposes for free.
  - Scores run e4m3 DoubleRow into [128,1024] PSUM tiles; exp outputs
    bf16 at FD=1024.  Row sums come from an all-ones [128,128]
    stationary matmul so they land in PSUM already broadcast across
    partitions; one DVE reciprocal yields bf16 1/rowsum, and the
    normalize multiplies run in the DVE 2x bf16 mode, emitting e4m3
    probabilities in [0,1] for the PV DoubleRow matmul.
"""

import sys
from contextlib import ExitStack

for _p in ("/opt/trn_rl_repo", "/root/.axon_site/_ro/trn_rl_repo"):
    if _p not in sys.path:
        sys.path.insert(0, _p)

import ml_dtypes
import numpy as np

import concourse.bacc as bacc
import concourse.bass as bass
import concourse.mybir as mybir
import concourse.tile as tile

P = 128
D = 512
DC = D // P   # contraction chunks of 128
CW = 1024     # DMA chunk width (columns)
INV_SQRT_D = float(1.0 / np.sqrt(D))
SQRT_D = float(np.sqrt(D))
EXP_SHIFT = -40.0

F32 = mybir.dt.float32
BF16 = mybir.dt.bfloat16
FP8 = mybir.dt.float8e4
AF = mybir.ActivationFunctionType
ALU = mybir.AluOpType
DR = mybir.MatmulPerfMode.DoubleRow


def build_program(nq, nkv, reps=1):
    nc = bacc.Bacc("TRN2", target_bir_lowering=False, debug=False, num_devices=8)

    NCH = nkv // CW
    NCHh = nq // CW
    # all host-pre-arranged, partition-major, fat contiguous runs
    xA8 = nc.dram_tensor("xA8", [P, NCH * DC * CW], FP8, kind="ExternalInput").ap()
    xB8 = nc.dram_tensor("xB8", [P, NCH * DC * CW], FP8, kind="ExternalInput").ap()
    xAh = nc.dram_tensor("xAh", [P, NCHh * DC * CW], BF16, kind="ExternalInput").ap()
    xBh = nc.dram_tensor("xBh", [P, NCHh * DC * CW], BF16, kind="ExternalInput").ap()
    wq8 = nc.dram_tensor("wq8", [P, DC * D], FP8, kind="ExternalInput").ap()
    wk8 = nc.dram_tensor("wk8", [P, DC * D], FP8, kind="ExternalInput").ap()
    wv8 = nc.dram_tensor("wv8", [P, DC * D], FP8, kind="ExternalInput").ap()
    wcA = nc.dram_tensor("wcA", [P, DC * D], BF16, kind="ExternalInput").ap()
    wcB = nc.dram_tensor("wcB", [P, DC * D], BF16, kind="ExternalInput").ap()
    # bq | bk | bc columns packed into one tensor: one tiny DMA, loaded
    # FIRST -- a late bias blocks the projection ACT drains, which holds
    # PSUM and stalls the PE for ~10us
    b3_pc = nc.dram_tensor("b3_pc", [P, 3 * DC], F32, kind="ExternalInput").ap()
    bv_row = nc.dram_tensor("bv_row", [1, D], BF16, kind="ExternalInput").ap()
    out = nc.dram_tensor("out", [D, nq], F32, kind="ExternalOutput").ap()

    with tile.TileContext(nc) as tc:
        for rep in range(reps):
            _emit_body(
                nc, tc, xA8, xB8, xAh, xBh, wq8, wk8, wv8, wcA, wcB,
                b3_pc, bv_row, out, nq, nkv, rep,
            )

    nc.compile()
    return nc


def _emit_body(
    nc, tc, xA8, xB8, xAh, xBh, wq8, wk8, wv8, wcA, wcB,
    b3_pc, bv_row, out, nq, nkv, rep,
):
    NBQ = nq // P   # query row blocks (16)
    MCK = nkv // P  # key row chunks (32)
    NCH = nkv // CW
    NCHh = nq // CW

    with ExitStack() as st:
        persist = st.enter_context(tc.tile_pool(name="persist", bufs=1))

        # ---------------- phase 1: projections (all local) ----------------
        with ExitStack() as ph1:
            xp = ph1.enter_context(tc.tile_pool(name="xp", bufs=1))
            ps1 = ph1.enter_context(tc.tile_pool(name="ps1", bufs=2, space="PSUM"))

            # x8/xh chunks alternate between the two HWDGE queues (sync +
            # scalar): each queue executes its DMAs serially incl. a ~2us
            # completion latency, so one-queue delivery starved the first
            # matmuls for ~11us.
            def _chunk(dram, ci):
                return dram[:, ci * DC * CW : (ci + 1) * DC * CW].rearrange(
                    "p (c w) -> p c w", c=DC
                )

            wk_sb = persist.tile([P, DC, D], FP8, name="w_wk")
            nc.sync.dma_start(out=wk_sb, in_=wk8.rearrange("p (c o) -> p c o", c=DC))
            b3_sb = persist.tile([P, 3 * DC], F32, name="b3_sb")
            nc.sync.dma_start(out=b3_sb, in_=b3_pc)
            bq_sb = b3_sb[:, 0:DC]
            bk_sb = b3_sb[:, DC : 2 * DC]
            bc_sb = b3_sb[:, 2 * DC : 3 * DC]
            wq_sb = persist.tile([P, DC, D], FP8, name="w_wq")
            nc.scalar.dma_start(out=wq_sb, in_=wq8.rearrange("p (c o) -> p c o", c=DC))
            bv_sb = persist.tile([1, D], BF16, name="bv_sb")
            nc.scalar.dma_start(out=bv_sb, in_=bv_row)
            xB8_sb = xp.tile([P, NCH, DC, CW], FP8, name="xB8_sb")
            xA8_sb = xp.tile([P, NCH, DC, CW], FP8, name="xA8_sb")
            for ci in range(NCH):
                eng = nc.sync if ci % 2 == 0 else nc.scalar
                eng.dma_start(out=xB8_sb[:, ci], in_=_chunk(xB8, ci))
            wv_sb = persist.tile([P, DC, D], FP8, name="w_wv")
            nc.scalar.dma_start(out=wv_sb, in_=wv8.rearrange("p (c o) -> p c o", c=DC))
            for ci in range(NCH):
                eng = nc.sync if ci % 2 == 1 else nc.scalar
                eng.dma_start(out=xA8_sb[:, ci], in_=_chunk(xA8, ci))

            wcA_sb = persist.tile([P, DC, D], BF16, name="w_wcA")
            nc.scalar.dma_start(out=wcA_sb, in_=wcA.rearrange("p (c o) -> p c o", c=DC))
            wcB_sb = persist.tile([P, DC, D], BF16, name="w_wcB")
            nc.sync.dma_start(out=wcB_sb, in_=wcB.rearrange("p (c o) -> p c o", c=DC))
            xAh_sb = xp.tile([P, NCHh, DC, CW], BF16, name="xAh_sb")
            xBh_sb = xp.tile([P, NCHh, DC, CW], BF16, name="xBh_sb")
            for ci in range(NCHh):
                nc.sync.dma_start(out=xAh_sb[:, ci], in_=_chunk(xAh, ci))
                nc.scalar.dma_start(out=xBh_sb[:, ci], in_=_chunk(xBh, ci))

            ones_row = persist.tile([1, 512], BF16, name="ones_row")
            nc.vector.memset(ones_row, 1.0)
            ones_mat = persist.tile([P, P], BF16, name="ones_mat")
            nc.vector.memset(ones_mat, 1.0)
            shift_sb = persist.tile([P, 1], F32, name="shift_sb")
            nc.vector.memset(shift_sb, EXP_SHIFT)

            # warm-up while the first DMAs land: junk matmuls hold the PE
            # HAM clock-gate at 2.4 GHz for the real work, and a 1-column
            # exp preloads the ACT table set (~2.7us otherwise paid right
            # before the first score tile).
            warm_act = persist.tile([P, 1], F32, name="warm_act")
            nc.scalar.activation(out=warm_act, in_=shift_sb, func=AF.Exp)
            warm_ps = ps1.tile([P, 2048], F32, name="ps1t", tag="ps1")
            for _wi in range(8):
                nc.tensor.matmul(
                    warm_ps[:, :512],
                    lhsT=ones_row[:, :P],
                    rhs=ones_row,
                    start=True,
                    stop=True,
                )

            qT_sb = persist.tile([P, DC, nq], FP8, name="qT_sb")
            kT_sb = persist.tile([P, DC, nkv], FP8, name="kT_sb")
            v_sb = persist.tile([P, MCK, D], FP8, name="v_sb")
            coT_sb = persist.tile([P, DC, nq], BF16, name="coT_sb")

            # K over all keys (fp8 DoubleRow, N=1024), then Q over own rows
            for s2 in range(0, nkv, 2048):
                for ob in range(DC):
                    ps = ps1.tile([P, 2048], F32, name="ps1t", tag="ps1")
                    for q4 in range(4):
                        s0 = s2 + q4 * 512
                        ci, off = s0 // CW, s0 % CW
                        for c2 in range(DC // 2):
                            nc.tensor.matmul(
                                ps[:, q4 * 512 : (q4 + 1) * 512],
                                lhsT=wk_sb[:, 2 * c2 : 2 * c2 + 2, ob * P : (ob + 1) * P],
                                rhs=xB8_sb[:, ci, 2 * c2 : 2 * c2 + 2, off : off + 512],
                                start=(c2 == 0),
                                stop=(c2 == DC // 2 - 1),
                                perf_mode=DR,
                            )
                    nc.scalar.activation(
                        out=kT_sb[:, ob, s2 : s2 + 2048],
                        in_=ps,
                        func=AF.Identity,
                        bias=bk_sb[:, ob : ob + 1],
                        scale=1.0,
                    )
            for ob in range(DC):
                ps = ps1.tile([P, 2048], F32, name="ps1t", tag="ps1")
                for q4 in range(4):
                    s0 = q4 * 512
                    ci, off = s0 // CW, s0 % CW
                    for c2 in range(DC // 2):
                        nc.tensor.matmul(
                            ps[:, q4 * 512 : (q4 + 1) * 512],
                            lhsT=wq_sb[:, 2 * c2 : 2 * c2 + 2, ob * P : (ob + 1) * P],
                            rhs=xA8_sb[:, ci, 2 * c2 : 2 * c2 + 2, off : off + 512],
                            start=(c2 == 0),
                            stop=(c2 == DC // 2 - 1),
                            perf_mode=DR,
                        )
                nc.scalar.activation(
                    out=qT_sb[:, ob, :],
                    in_=ps,
                    func=AF.Identity,
                    bias=bq_sb[:, ob : ob + 1],
                    scale=1.0,
                )

            # V over all key rows of the A view (fp8 DoubleRow), e4m3 out;
            # emitted before co so the PE has work while co's bf16 x
            # halves are still streaming in
            for mg in range(0, MCK, 4):
                ps = ps1.tile([P, 2048], F32, name="ps1t", tag="ps1")
                for mi in range(4):
                    m = mg + mi
                    ci, off = (m * P) // CW, (m * P) % CW
                    for c2 in range(DC // 2):
                        nc.tensor.matmul(
                            ps[:, mi * 512 : (mi + 1) * 512],
                            lhsT=xA8_sb[:, ci, 2 * c2 : 2 * c2 + 2, off : off + P],
                            rhs=wv_sb[:, 2 * c2 : 2 * c2 + 2, :],
                            start=(c2 == 0),
                            stop=(c2 == DC // 2 - 1),
                            perf_mode=DR,
                        )
                nc.scalar.activation(out=v_sb[:, mg : mg + 4, :], in_=ps, func=AF.Copy)

            # co^T = relu(WcA^T xA + WcB^T xB + bc): transposed (feature on
            # partitions) so bc+relu fuse into the ACT drain per-partition
            for ob in range(DC):
                ps = ps1.tile([P, 2048], F32, name="ps1t", tag="ps1")
                for q4 in range(4):
                    s0 = q4 * 512
                    ci, off = s0 // CW, s0 % CW
                    first = True
                    for w_sb, xh_sb in ((wcA_sb, xAh_sb), (wcB_sb, xBh_sb)):
                        for c in range(DC):
                            last = w_sb is wcB_sb and c == DC - 1
                            nc.tensor.matmul(
                                ps[:, q4 * 512 : (q4 + 1) * 512],
                                lhsT=w_sb[:, c, ob * P : (ob + 1) * P],
                                rhs=xh_sb[:, ci, c, off : off + 512],
                                start=first,
                                stop=last,
                            )
                            first = False
                nc.scalar.activation(
                    out=coT_sb[:, ob, :],
                    in_=ps,
                    func=AF.Relu,
                    bias=bc_sb[:, ob : ob + 1],
                    scale=1.0,
                )

        # ---------------- phase 2: attention (S^T layout) ----------------
        at_pool = st.enter_context(tc.tile_pool(name="at_pool", bufs=2))
        a8_pool = st.enter_context(tc.tile_pool(name="a8_pool", bufs=2))
        r_pool = st.enter_context(tc.tile_pool(name="r_pool", bufs=2))
        o_pool = st.enter_context(tc.tile_pool(name="o_pool", bufs=4))
        sps_pool = st.enter_context(tc.tile_pool(name="sps", bufs=2, space="PSUM"))
        sum_pool = st.enter_context(tc.tile_pool(name="sump", bufs=2, space="PSUM"))
        pv_pool = st.enter_context(tc.tile_pool(name="pv", bufs=2, space="PSUM"))

        for s0 in range(0, nq, 512):
            at_sb = at_pool.tile([P, MCK, 512], BF16, name="at_sb", tag="at")
            at8 = a8_pool.tile([P, MCK, 512], FP8, name="at8", tag="at8")
            # rowsum, broadcast across partitions by the all-ones lhsT
            ssum = sum_pool.tile([P, 512], F32, name="ssum", tag="ssum")
            for mbp in range(MCK // 2):
                sps = sps_pool.tile([P, 1024], F32, name="sps", tag="sps")
                for mi in range(2):
                    mb = 2 * mbp + mi
                    for c2 in range(DC // 2):
                        nc.tensor.matmul(
                            sps[:, mi * 512 : (mi + 1) * 512],
                            lhsT=kT_sb[:, 2 * c2 : 2 * c2 + 2, mb * P : (mb + 1) * P],
                            rhs=qT_sb[:, 2 * c2 : 2 * c2 + 2, s0 : s0 + 512],
                            start=(c2 == 0),
                            stop=(c2 == DC // 2 - 1),
                            perf_mode=DR,
                        )
                nc.scalar.activation(
                    out=at_sb[:, 2 * mbp : 2 * mbp + 2, :],
                    in_=sps,
                    func=AF.Exp,
                    bias=shift_sb,
                    scale=1.0,
                )
                for mi in range(2):
                    mb = 2 * mbp + mi
                    nc.tensor.matmul(
                        ssum,
                        lhsT=ones_mat,
                        rhs=at_sb[:, mb, :],
                        start=(mb == 0),
                        stop=(mb == MCK - 1),
                    )
            # 1/rowsum (already broadcast).  The exact DVE reciprocal is an
            # 8-cycle/element iterative divide (~3.4us, head-of-line blocks
            # the PE); approx_fast is ~51 ULP -- far below the bf16
            # rounding of rs_bc that the normalize already accepts.
            # rowsum >= exp(-40+max_score) stays well above denormals.
            rs_f32 = r_pool.tile([P, 512], F32, name="rs_f32", tag="rsf")
            nc.vector.reciprocal_approx_fast(out=rs_f32, in_=ssum)
            rs_bc = r_pool.tile([P, 512], BF16, name="rs_bc", tag="rsb")
            with nc.allow_low_precision(reason="bf16 1/rowsum: 0.4% on a term ~5% of |out|"):
                nc.vector.tensor_copy(rs_bc, rs_f32)
            for mb in range(MCK):
                nc.vector.tensor_mul(at8[:, mb, :], at_sb[:, mb, :], rs_bc)

            # PV^T: partitions = output feature; bv enters the PSUM group
            # via ones x bv (rows sum to 1 -> lands as +bv/sqrt(D))
            for j in range(DC):
                pv = pv_pool.tile([P, 512], F32, name="pv", tag="pv")
                nc.tensor.matmul(
                    pv,
                    lhsT=bv_sb[:, j * P : (j + 1) * P],
                    rhs=ones_row,
                    start=True,
                    stop=False,
                )
                for i2 in range(MCK // 2):
                    nc.tensor.matmul(
                        pv,
                        lhsT=v_sb[:, 2 * i2 : 2 * i2 + 2, j * P : (j + 1) * P],
                        rhs=at8[:, 2 * i2 : 2 * i2 + 2, :],
                        start=False,
                        stop=(i2 == MCK // 2 - 1),
                        perf_mode=DR,
                    )
                outt = o_pool.tile([P, 512], F32, name="outt", tag="outt")
                nc.vector.scalar_tensor_tensor(
                    out=outt,
                    in0=pv,
                    scalar=INV_SQRT_D,
                    in1=coT_sb[:, j, s0 : s0 + 512],
                    op0=ALU.mult,
                    op1=ALU.add,
                )
                nc.sync.dma_start(
                    out=out[j * P : (j + 1) * P, s0 : s0 + 512], in_=outt
                )


_PROG_CACHE = {}


def _get_program(nq, nkv):
    key = (nq, nkv)
    if key not in _PROG_CACHE:
        _PROG_CACHE[key] = build_program(nq, nkv)
    return _PROG_CACHE[key]


def _pc_chunks(xT, dt):
    """[D, N] feature-major -> [P, NCH*DC*CW] partition-major fat runs."""
    Dd, N = xT.shape
    nch = N // CW
    return np.ascontiguousarray(
        xT.reshape(DC, P, nch, CW).transpose(1, 2, 0, 3).reshape(P, -1)
    ).astype(dt)


def _pc_weight(WT, dt):
    """[D, D] (in, out) -> [P, DC*D] partition-major."""
    return np.ascontiguousarray(
        WT.reshape(DC, P, D).transpose(1, 0, 2).reshape(P, -1)
    ).astype(dt)


def make_in_maps(x_f, x_s, Wq, bq, Wk, bk, Wv, bv, Wc, bc):
    """Per-core SPMD input dicts + (direction, batch, half) layout."""
    x_f = np.asarray(x_f, np.float32)
    x_s = np.asarray(x_s, np.float32)
    B, N, _ = x_f.shape
    nq = N // 2
    bf = ml_dtypes.bfloat16
    e4 = ml_dtypes.float8_e4m3
    Wq8 = _pc_weight(np.asarray(Wq, np.float32).T, e4)
    Wk8 = _pc_weight(np.asarray(Wk, np.float32).T, e4)
    Wv8 = _pc_weight(np.asarray(Wv, np.float32).T, e4)
    Wc = np.asarray(Wc, np.float32)
    WcfT = _pc_weight(np.ascontiguousarray(Wc[:, :D].T), bf)
    WcsT = _pc_weight(np.ascontiguousarray(Wc[:, D:].T), bf)
    bq32, bk32, bv32, bc32 = (
        np.asarray(b, np.float32) for b in (bq, bk, bv, bc)
    )
    b3_pc = np.ascontiguousarray(
        np.concatenate(
            [b.reshape(DC, P).T for b in (bq32, bk32, bc32)], axis=1
        )
    )
    # rows sum to 1 after L1-normalize, so +bv in the PV PSUM lands as
    # +bv/sqrt(D) in the output after the epilogue's 1/sqrt(D) scale --
    # exactly the reference's V-projection bias term
    bv_row = np.ascontiguousarray(bv32[None, :]).astype(bf)
    xT_f = [np.ascontiguousarray(x_f[b].T) for b in range(B)]
    xT_s = [np.ascontiguousarray(x_s[b].T) for b in range(B)]
    in_maps, layout = [], []
    for d in range(2):
        for b in range(B):
            for h in range(2):
                xq = xT_f[b] if d == 0 else xT_s[b]
                xk = xT_s[b] if d == 0 else xT_f[b]
                if h == 1:
                    idx = np.r_[nq:N, 0:nq]
                    xq, xk = xq[:, idx], xk[:, idx]
                in_maps.append(
                    {
                        "xA8": _pc_chunks(xq, e4),
                        "xB8": _pc_chunks(xk, e4),
                        "xAh": _pc_chunks(xq[:, :nq], bf),
                        "xBh": _pc_chunks(xk[:, :nq], bf),
                        "wq8": Wq8,
                        "wk8": Wk8,
                        "wv8": Wv8,
                        "wcA": WcfT if d == 0 else WcsT,
                        "wcB": WcsT if d == 0 else WcfT,
                        "b3_pc": b3_pc,
                        "bv_row": bv_row,
                    }
                )
                layout.append((d, b, h))
    return in_maps, layout


def kernel(x_f, x_s, Wq, bq, Wk, bk, Wv, bv, Wc, bc):
    x_f = np.asarray(x_f, np.float32)
    B, N, _ = x_f.shape
    nq = N // 2
    nc = _get_program(nq, N)
    in_maps, layout = make_in_maps(x_f, x_s, Wq, bq, Wk, bk, Wv, bv, Wc, bc)

    from concourse.bass_utils import run_bass_kernel_spmd

    res = run_bass_kernel_spmd(nc, in_maps, list(range(len(in_maps))))
    out_f = np.empty((B, N, D), np.float32)
    out_s = np.empty((B, N, D), np.float32)
    for (d, b, h), r in zip(layout, res.results):
        tgt = out_f if d == 0 else out_s
        tgt[b, h * nq : (h + 1) * nq] = r["out"].T
    return out_f, out_s


# revision 25
# speedup vs baseline: 1.0299x; 1.0299x over previous
"""Cross-view attention (nn_CrossViewAttention) Trainium2 Bass kernel.

Reference computation (B=2, N=4096, D=512):
    co    = relu(concat([x_f, x_s], -1) @ Wc.T + bc)
    out_f = attend(x_f@Wq.T+bq, x_s@Wk.T+bk, x_f@Wv.T+bv) + co
    out_s = attend(x_s@Wq.T+bq, x_f@Wk.T+bk, x_s@Wv.T+bv) + co
    attend(Q,K,V) = (softmax(Q K^T) / L1 / sqrt(D)) @ V

Sharding: 8 cores = (direction f/s) x (batch 0/1) x (sequence half).
Each core computes 2048 output rows of one direction against the full
4096-row K/V for its (direction, batch), SPMD with per-core input data.
Rows are permuted host-side so every core's own rows come first; the
attention reduction over keys is permutation invariant.  Collectives
proved to serialize the whole kernel for a minor PE saving, so K/V/co
are computed fully locally.

Design notes (v3):
  - Q/K/V projections run in fp8-e4m3 DoubleRow (host ships e4m3 copies
    of x^T and the weights); the co MLP stays bf16 (co dominates the
    output magnitude; fp8 there fails the accuracy gate).  bf16 x ships
    only for the core's own 2048 rows (co's input).
  - Every DMA-able tensor is pre-arranged on the host so each partition
    reads one fat contiguous run (4-16 KiB descriptors); the v2 layout
    produced 0.5-1 KiB descriptors that starved the first matmuls.
  - Input DMAs split across the two HWDGE queues (sync carries the K/Q
    operands, scalar everything else) so the first matmuls start right
    after the NEFF preamble.
  - Projections accumulate into [128,2048] PSUM tiles (4 banks, one
    8-bank pool of 2) and drain with FD=2048 ACT ops, so ACT no longer
    paces the PE during phase 1 (v2 lost ~146ns per 2-matmul group).
  - co and PV produce TRANSPOSED tiles (partitions = output feature):
    bc+relu fuse into the co ACT drain as a per-partition bias, bv
    enters the PV PSUM group via a ones-row matmul (rows sum to 1 after
    L1-normalize, so +bv there lands as +bv/sqrt(D) in the output), and
    the epilogue stays one fused DVE op out^T = pv^T/sqrt(D) + co^T.
    The kernel emits out^T [D, nq]; the host transposes for free.
  - Scores run e4m3 DoubleRow into [128,1024] PSUM tiles; exp outputs
    bf16 at FD=1024.  Row sums come from an all-ones [128,128]
    stationary matmul so they land in PSUM already broadcast across
    partitions; one DVE reciprocal yields bf16 1/rowsum, and the
    normalize multiplies run in the DVE 2x bf16 mode, emitting e4m3
    probabilities in [0,1] for the PV DoubleRow matmul.
"""

import sys
from contextlib import ExitStack

for _p in ("/opt/trn_rl_repo", "/root/.axon_site/_ro/trn_rl_repo"):
    if _p not in sys.path:
        sys.path.insert(0, _p)

import ml_dtypes
import numpy as np

import concourse.bacc as bacc
import concourse.bass as bass
import concourse.mybir as mybir
import concourse.tile as tile

P = 128
D = 512
DC = D // P   # contraction chunks of 128
CW = 1024     # DMA chunk width (columns)
INV_SQRT_D = float(1.0 / np.sqrt(D))
SQRT_D = float(np.sqrt(D))
EXP_SHIFT = -40.0

F32 = mybir.dt.float32
BF16 = mybir.dt.bfloat16
FP8 = mybir.dt.float8e4
AF = mybir.ActivationFunctionType
ALU = mybir.AluOpType
DR = mybir.MatmulPerfMode.DoubleRow


def build_program(nq, nkv, reps=1):
    nc = bacc.Bacc("TRN2", target_bir_lowering=False, debug=False, num_devices=8)

    NCH = nkv // CW
    NCHh = nq // CW
    # all host-pre-arranged, partition-major, fat contiguous runs
    xA8 = nc.dram_tensor("xA8", [P, NCH * DC * CW], FP8, kind="ExternalInput").ap()
    xB8 = nc.dram_tensor("xB8", [P, NCH * DC * CW], FP8, kind="ExternalInput").ap()
    xAh = nc.dram_tensor("xAh", [P, NCHh * DC * CW], BF16, kind="ExternalInput").ap()
    xBh = nc.dram_tensor("xBh", [P, NCHh * DC * CW], BF16, kind="ExternalInput").ap()
    wq8 = nc.dram_tensor("wq8", [P, DC * D], FP8, kind="ExternalInput").ap()
    wk8 = nc.dram_tensor("wk8", [P, DC * D], FP8, kind="ExternalInput").ap()
    wv8 = nc.dram_tensor("wv8", [P, DC * D], FP8, kind="ExternalInput").ap()
    wcA = nc.dram_tensor("wcA", [P, DC * D], BF16, kind="ExternalInput").ap()
    wcB = nc.dram_tensor("wcB", [P, DC * D], BF16, kind="ExternalInput").ap()
    # bq | bk | bc columns packed into one tensor: one tiny DMA, loaded
    # FIRST -- a late bias blocks the projection ACT drains, which holds
    # PSUM and stalls the PE for ~10us
    b3_pc = nc.dram_tensor("b3_pc", [P, 3 * DC], F32, kind="ExternalInput").ap()
    bv_row = nc.dram_tensor("bv_row", [1, D], BF16, kind="ExternalInput").ap()
    out = nc.dram_tensor("out", [D, nq], F32, kind="ExternalOutput").ap()

    with tile.TileContext(nc) as tc:
        for rep in range(reps):
            _emit_body(
                nc, tc, xA8, xB8, xAh, xBh, wq8, wk8, wv8, wcA, wcB,
                b3_pc, bv_row, out, nq, nkv, rep,
            )

    nc.compile()
    return nc


def _emit_body(
    nc, tc, xA8, xB8, xAh, xBh, wq8, wk8, wv8, wcA, wcB,
    b3_pc, bv_row, out, nq, nkv, rep,
):
    NBQ = nq // P   # query row blocks (16)
    MCK = nkv // P  # key row chunks (32)
    NCH = nkv // CW
    NCHh = nq // CW

    with ExitStack() as st:
        persist = st.enter_context(tc.tile_pool(name="persist", bufs=1))

        # ---------------- phase 1: projections (all local) ----------------
        with ExitStack() as ph1:
            xp = ph1.enter_context(tc.tile_pool(name="xp", bufs=1))
            ps1 = ph1.enter_context(tc.tile_pool(name="ps1", bufs=3, space="PSUM"))
            wmp = ph1.enter_context(tc.tile_pool(name="wmp", bufs=1, space="PSUM"))

            # x8/xh chunks alternate between the two HWDGE queues (sync +
            # scalar): each queue executes its DMAs serially incl. a ~2us
            # completion latency, so one-queue delivery starved the first
            # matmuls for ~11us.
            def _chunk(dram, ci):
                return dram[:, ci * DC * CW : (ci + 1) * DC * CW].rearrange(
                    "p (c w) -> p c w", c=DC
                )

            wk_sb = persist.tile([P, DC, D], FP8, name="w_wk")
            nc.sync.dma_start(out=wk_sb, in_=wk8.rearrange("p (c o) -> p c o", c=DC))
            b3_sb = persist.tile([P, 3 * DC], F32, name="b3_sb")
            nc.sync.dma_start(out=b3_sb, in_=b3_pc)
            bq_sb = b3_sb[:, 0:DC]
            bk_sb = b3_sb[:, DC : 2 * DC]
            bc_sb = b3_sb[:, 2 * DC : 3 * DC]
            wq_sb = persist.tile([P, DC, D], FP8, name="w_wq")
            nc.scalar.dma_start(out=wq_sb, in_=wq8.rearrange("p (c o) -> p c o", c=DC))
            bv_sb = persist.tile([1, D], BF16, name="bv_sb")
            nc.scalar.dma_start(out=bv_sb, in_=bv_row)
            xB8_sb = xp.tile([P, NCH, DC, CW], FP8, name="xB8_sb")
            xA8_sb = xp.tile([P, NCH, DC, CW], FP8, name="xA8_sb")
            for ci in range(NCH):
                eng = nc.sync if ci % 2 == 0 else nc.scalar
                eng.dma_start(out=xB8_sb[:, ci], in_=_chunk(xB8, ci))
            wv_sb = persist.tile([P, DC, D], FP8, name="w_wv")
            nc.scalar.dma_start(out=wv_sb, in_=wv8.rearrange("p (c o) -> p c o", c=DC))
            for ci in range(NCH):
                eng = nc.sync if ci % 2 == 1 else nc.scalar
                eng.dma_start(out=xA8_sb[:, ci], in_=_chunk(xA8, ci))

            wcA_sb = persist.tile([P, DC, D], BF16, name="w_wcA")
            nc.scalar.dma_start(out=wcA_sb, in_=wcA.rearrange("p (c o) -> p c o", c=DC))
            wcB_sb = persist.tile([P, DC, D], BF16, name="w_wcB")
            nc.sync.dma_start(out=wcB_sb, in_=wcB.rearrange("p (c o) -> p c o", c=DC))
            xAh_sb = xp.tile([P, NCHh, DC, CW], BF16, name="xAh_sb")
            xBh_sb = xp.tile([P, NCHh, DC, CW], BF16, name="xBh_sb")
            for ci in range(NCHh):
                nc.sync.dma_start(out=xAh_sb[:, ci], in_=_chunk(xAh, ci))
                nc.scalar.dma_start(out=xBh_sb[:, ci], in_=_chunk(xBh, ci))

            ones_row = persist.tile([1, 512], BF16, name="ones_row")
            nc.vector.memset(ones_row, 1.0)
            ones_mat = persist.tile([P, P], BF16, name="ones_mat")
            nc.vector.memset(ones_mat, 1.0)
            shift_sb = persist.tile([P, 1], F32, name="shift_sb")
            nc.vector.memset(shift_sb, EXP_SHIFT)

            # warm-up while the first DMAs land: junk matmuls hold the PE
            # HAM clock-gate at 2.4 GHz for the real work, and a 1-column
            # exp preloads the ACT table set (~2.7us otherwise paid right
            # before the first score tile).
            warm_act = persist.tile([P, 1], F32, name="warm_act")
            nc.scalar.activation(out=warm_act, in_=shift_sb, func=AF.Exp)
            warm_ps = wmp.tile([P, 512], F32, name="warm_ps")
            for _wi in range(8):
                nc.tensor.matmul(
                    warm_ps,
                    lhsT=ones_row[:, :P],
                    rhs=ones_row,
                    start=True,
                    stop=True,
                )

            qT_sb = persist.tile([P, DC, nq], FP8, name="qT_sb")
            kT_sb = persist.tile([P, DC, nkv], FP8, name="kT_sb")
            v_sb = persist.tile([P, MCK, D], FP8, name="v_sb")
            coT_sb = persist.tile([P, DC, nq], BF16, name="coT_sb")

            def _junk(n):
                # junk matmuls into the dedicated warm bank: keep the PE
                # HAM window busy across DMA-delivery hiccups -- a >3.4us
                # idle re-throttles the clock to 1.2 GHz and every matmul
                # after runs at half rate for the next ~3.4us
                for _wi in range(n):
                    nc.tensor.matmul(
                        warm_ps,
                        lhsT=ones_row[:, :P],
                        rhs=ones_row,
                        start=True,
                        stop=True,
                    )

            # K over all keys (fp8 DoubleRow), then Q over own rows
            for s1 in range(0, nkv, 1024):
                for ob in range(DC):
                    ps = ps1.tile([P, 1024], F32, name="ps1t", tag="ps1")
                    for q4 in range(2):
                        s0 = s1 + q4 * 512
                        ci, off = s0 // CW, s0 % CW
                        for c2 in range(DC // 2):
                            nc.tensor.matmul(
                                ps[:, q4 * 512 : (q4 + 1) * 512],
                                lhsT=wk_sb[:, 2 * c2 : 2 * c2 + 2, ob * P : (ob + 1) * P],
                                rhs=xB8_sb[:, ci, 2 * c2 : 2 * c2 + 2, off : off + 512],
                                start=(c2 == 0),
                                stop=(c2 == DC // 2 - 1),
                                perf_mode=DR,
                            )
                    nc.scalar.activation(
                        out=kT_sb[:, ob, s1 : s1 + 1024],
                        in_=ps,
                        func=AF.Identity,
                        bias=bk_sb[:, ob : ob + 1],
                        scale=1.0,
                    )
                    if s1 <= 1024:
                        _junk(2)
            for s1 in range(0, nq, 1024):
                for ob in range(DC):
                    ps = ps1.tile([P, 1024], F32, name="ps1t", tag="ps1")
                    for q4 in range(2):
                        s0 = s1 + q4 * 512
                        ci, off = s0 // CW, s0 % CW
                        for c2 in range(DC // 2):
                            nc.tensor.matmul(
                                ps[:, q4 * 512 : (q4 + 1) * 512],
                                lhsT=wq_sb[:, 2 * c2 : 2 * c2 + 2, ob * P : (ob + 1) * P],
                                rhs=xA8_sb[:, ci, 2 * c2 : 2 * c2 + 2, off : off + 512],
                                start=(c2 == 0),
                                stop=(c2 == DC // 2 - 1),
                                perf_mode=DR,
                            )
                    nc.scalar.activation(
                        out=qT_sb[:, ob, s1 : s1 + 1024],
                        in_=ps,
                        func=AF.Identity,
                        bias=bq_sb[:, ob : ob + 1],
                        scale=1.0,
                    )

            # V over all key rows of the A view (fp8 DoubleRow), e4m3 out;
            # emitted before co so the PE has work while co's bf16 x
            # halves are still streaming in
            for mg in range(0, MCK, 2):
                ps = ps1.tile([P, 1024], F32, name="ps1t", tag="ps1")
                for mi in range(2):
                    m = mg + mi
                    ci, off = (m * P) // CW, (m * P) % CW
                    for c2 in range(DC // 2):
                        nc.tensor.matmul(
                            ps[:, mi * 512 : (mi + 1) * 512],
                            lhsT=xA8_sb[:, ci, 2 * c2 : 2 * c2 + 2, off : off + P],
                            rhs=wv_sb[:, 2 * c2 : 2 * c2 + 2, :],
                            start=(c2 == 0),
                            stop=(c2 == DC // 2 - 1),
                            perf_mode=DR,
                        )
                nc.scalar.activation(out=v_sb[:, mg : mg + 2, :], in_=ps, func=AF.Copy)

            # co^T = relu(WcA^T xA + WcB^T xB + bc): transposed (feature on
            # partitions) so bc+relu fuse into the ACT drain per-partition
            for ob in range(DC):
                for s1 in range(0, nq, 1024):
                    ps = ps1.tile([P, 1024], F32, name="ps1t", tag="ps1")
                    for q4 in range(2):
                        s0 = s1 + q4 * 512
                        ci, off = s0 // CW, s0 % CW
                        first = True
                        for w_sb, xh_sb in ((wcA_sb, xAh_sb), (wcB_sb, xBh_sb)):
                            for c in range(DC):
                                last = w_sb is wcB_sb and c == DC - 1
                                nc.tensor.matmul(
                                    ps[:, q4 * 512 : (q4 + 1) * 512],
                                    lhsT=w_sb[:, c, ob * P : (ob + 1) * P],
                                    rhs=xh_sb[:, ci, c, off : off + 512],
                                    start=first,
                                    stop=last,
                                )
                                first = False
                    nc.scalar.activation(
                        out=coT_sb[:, ob, s1 : s1 + 1024],
                        in_=ps,
                        func=AF.Relu,
                        bias=bc_sb[:, ob : ob + 1],
                        scale=1.0,
                    )

        # ---------------- phase 2: attention (S^T layout) ----------------
        at_pool = st.enter_context(tc.tile_pool(name="at_pool", bufs=2))
        a8_pool = st.enter_context(tc.tile_pool(name="a8_pool", bufs=2))
        r_pool = st.enter_context(tc.tile_pool(name="r_pool", bufs=2))
        o_pool = st.enter_context(tc.tile_pool(name="o_pool", bufs=4))
        sps_pool = st.enter_context(tc.tile_pool(name="sps", bufs=2, space="PSUM"))
        sum_pool = st.enter_context(tc.tile_pool(name="sump", bufs=1, space="PSUM"))
        pv_pool = st.enter_context(tc.tile_pool(name="pv", bufs=3, space="PSUM"))

        for s0 in range(0, nq, 512):
            at_sb = at_pool.tile([P, MCK, 512], BF16, name="at_sb", tag="at")
            at8 = a8_pool.tile([P, MCK, 512], FP8, name="at8", tag="at8")
            # rowsum, broadcast across partitions by the all-ones lhsT
            ssum = sum_pool.tile([P, 512], F32, name="ssum", tag="ssum")
            for mbp in range(MCK // 2):
                sps = sps_pool.tile([P, 1024], F32, name="sps", tag="sps")
                for mi in range(2):
                    mb = 2 * mbp + mi
                    for c2 in range(DC // 2):
                        nc.tensor.matmul(
                            sps[:, mi * 512 : (mi + 1) * 512],
                            lhsT=kT_sb[:, 2 * c2 : 2 * c2 + 2, mb * P : (mb + 1) * P],
                            rhs=qT_sb[:, 2 * c2 : 2 * c2 + 2, s0 : s0 + 512],
                            start=(c2 == 0),
                            stop=(c2 == DC // 2 - 1),
                            perf_mode=DR,
                        )
                nc.scalar.activation(
                    out=at_sb[:, 2 * mbp : 2 * mbp + 2, :],
                    in_=sps,
                    func=AF.Exp,
                    bias=shift_sb,
                    scale=1.0,
                )
                for mi in range(2):
                    mb = 2 * mbp + mi
                    nc.tensor.matmul(
                        ssum,
                        lhsT=ones_mat,
                        rhs=at_sb[:, mb, :],
                        start=(mb == 0),
                        stop=(mb == MCK - 1),
                    )
            # 1/rowsum (already broadcast).  The exact DVE reciprocal is an
            # 8-cycle/element iterative divide (~3.4us, head-of-line blocks
            # the PE); approx_fast is ~51 ULP -- far below the bf16
            # rounding of rs_bc that the normalize already accepts.
            # rowsum >= exp(-40+max_score) stays well above denormals.
            rs_f32 = r_pool.tile([P, 512], F32, name="rs_f32", tag="rsf")
            nc.vector.reciprocal_approx_fast(out=rs_f32, in_=ssum)
            rs_bc = r_pool.tile([P, 512], BF16, name="rs_bc", tag="rsb")
            with nc.allow_low_precision(reason="bf16 1/rowsum: 0.4% on a term ~5% of |out|"):
                nc.vector.tensor_copy(rs_bc, rs_f32)
            for mb in range(MCK):
                nc.vector.tensor_mul(at8[:, mb, :], at_sb[:, mb, :], rs_bc)

            # PV^T: partitions = output feature; bv enters the PSUM group
            # via ones x bv (rows sum to 1 -> lands as +bv/sqrt(D))
            for j in range(DC):
                pv = pv_pool.tile([P, 512], F32, name="pv", tag="pv")
                nc.tensor.matmul(
                    pv,
                    lhsT=bv_sb[:, j * P : (j + 1) * P],
                    rhs=ones_row,
                    start=True,
                    stop=False,
                )
                for i2 in range(MCK // 2):
                    nc.tensor.matmul(
                        pv,
                        lhsT=v_sb[:, 2 * i2 : 2 * i2 + 2, j * P : (j + 1) * P],
                        rhs=at8[:, 2 * i2 : 2 * i2 + 2, :],
                        start=False,
                        stop=(i2 == MCK // 2 - 1),
                        perf_mode=DR,
                    )
                outt = o_pool.tile([P, 512], F32, name="outt", tag="outt")
                nc.vector.scalar_tensor_tensor(
                    out=outt,
                    in0=pv,
                    scalar=INV_SQRT_D,
                    in1=coT_sb[:, j, s0 : s0 + 512],
                    op0=ALU.mult,
                    op1=ALU.add,
                )
                nc.sync.dma_start(
                    out=out[j * P : (j + 1) * P, s0 : s0 + 512], in_=outt
                )


_PROG_CACHE = {}


def _get_program(nq, nkv):
    key = (nq, nkv)
    if key not in _PROG_CACHE:
        _PROG_CACHE[key] = build_program(nq, nkv)
    return _PROG_CACHE[key]


def _pc_chunks(xT, dt):
    """[D, N] feature-major -> [P, NCH*DC*CW] partition-major fat runs."""
    Dd, N = xT.shape
    nch = N // CW
    return np.ascontiguousarray(
        xT.reshape(DC, P, nch, CW).transpose(1, 2, 0, 3).reshape(P, -1)
    ).astype(dt)


def _pc_weight(WT, dt):
    """[D, D] (in, out) -> [P, DC*D] partition-major."""
    return np.ascontiguousarray(
        WT.reshape(DC, P, D).transpose(1, 0, 2).reshape(P, -1)
    ).astype(dt)


def make_in_maps(x_f, x_s, Wq, bq, Wk, bk, Wv, bv, Wc, bc):
    """Per-core SPMD input dicts + (direction, batch, half) layout."""
    x_f = np.asarray(x_f, np.float32)
    x_s = np.asarray(x_s, np.float32)
    B, N, _ = x_f.shape
    nq = N // 2
    bf = ml_dtypes.bfloat16
    e4 = ml_dtypes.float8_e4m3
    Wq8 = _pc_weight(np.asarray(Wq, np.float32).T, e4)
    Wk8 = _pc_weight(np.asarray(Wk, np.float32).T, e4)
    Wv8 = _pc_weight(np.asarray(Wv, np.float32).T, e4)
    Wc = np.asarray(Wc, np.float32)
    WcfT = _pc_weight(np.ascontiguousarray(Wc[:, :D].T), bf)
    WcsT = _pc_weight(np.ascontiguousarray(Wc[:, D:].T), bf)
    bq32, bk32, bv32, bc32 = (
        np.asarray(b, np.float32) for b in (bq, bk, bv, bc)
    )
    b3_pc = np.ascontiguousarray(
        np.concatenate(
            [b.reshape(DC, P).T for b in (bq32, bk32, bc32)], axis=1
        )
    )
    # rows sum to 1 after L1-normalize, so +bv in the PV PSUM lands as
    # +bv/sqrt(D) in the output after the epilogue's 1/sqrt(D) scale --
    # exactly the reference's V-projection bias term
    bv_row = np.ascontiguousarray(bv32[None, :]).astype(bf)
    xT_f = [np.ascontiguousarray(x_f[b].T) for b in range(B)]
    xT_s = [np.ascontiguousarray(x_s[b].T) for b in range(B)]
    in_maps, layout = [], []
    for d in range(2):
        for b in range(B):
            for h in range(2):
                xq = xT_f[b] if d == 0 else xT_s[b]
                xk = xT_s[b] if d == 0 else xT_f[b]
                if h == 1:
                    idx = np.r_[nq:N, 0:nq]
                    xq, xk = xq[:, idx], xk[:, idx]
                in_maps.append(
                    {
                        "xA8": _pc_chunks(xq, e4),
                        "xB8": _pc_chunks(xk, e4),
                        "xAh": _pc_chunks(xq[:, :nq], bf),
                        "xBh": _pc_chunks(xk[:, :nq], bf),
                        "wq8": Wq8,
                        "wk8": Wk8,
                        "wv8": Wv8,
                        "wcA": WcfT if d == 0 else WcsT,
                        "wcB": WcsT if d == 0 else WcfT,
                        "b3_pc": b3_pc,
                        "bv_row": bv_row,
                    }
                )
                layout.append((d, b, h))
    return in_maps, layout


def kernel(x_f, x_s, Wq, bq, Wk, bk, Wv, bv, Wc, bc):
    x_f = np.asarray(x_f, np.float32)
    B, N, _ = x_f.shape
    nq = N // 2
    nc = _get_program(nq, N)
    in_maps, layout = make_in_maps(x_f, x_s, Wq, bq, Wk, bk, Wv, bv, Wc, bc)

    from concourse.bass_utils import run_bass_kernel_spmd

    res = run_bass_kernel_spmd(nc, in_maps, list(range(len(in_maps))))
    out_f = np.empty((B, N, D), np.float32)
    out_s = np.empty((B, N, D), np.float32)
    for (d, b, h), r in zip(layout, res.results):
        tgt = out_f if d == 0 else out_s
        tgt[b, h * nq : (h + 1) * nq] = r["out"].T
    return out_f, out_s


# revision 28
# speedup vs baseline: 1.0527x; 1.0222x over previous
"""Cross-view attention (nn_CrossViewAttention) Trainium2 Bass kernel.

Reference computation (B=2, N=4096, D=512):
    co    = relu(concat([x_f, x_s], -1) @ Wc.T + bc)
    out_f = attend(x_f@Wq.T+bq, x_s@Wk.T+bk, x_f@Wv.T+bv) + co
    out_s = attend(x_s@Wq.T+bq, x_f@Wk.T+bk, x_s@Wv.T+bv) + co
    attend(Q,K,V) = (softmax(Q K^T) / L1 / sqrt(D)) @ V

Sharding: 8 cores = (direction f/s) x (batch 0/1) x (sequence half).
Each core computes 2048 output rows of one direction against the full
4096-row K/V for its (direction, batch), SPMD with per-core input data.
Rows are permuted host-side so every core's own rows come first; the
attention reduction over keys is permutation invariant.  Collectives
proved to serialize the whole kernel for a minor PE saving, so K/V/co
are computed fully locally.

Design notes (v3):
  - Q/K/V projections run in fp8-e4m3 DoubleRow (host ships e4m3 copies
    of x^T and the weights); the co MLP stays bf16 (co dominates the
    output magnitude; fp8 there fails the accuracy gate).  bf16 x ships
    only for the core's own 2048 rows (co's input).
  - Every DMA-able tensor is pre-arranged on the host so each partition
    reads one fat contiguous run (4-16 KiB descriptors); the v2 layout
    produced 0.5-1 KiB descriptors that starved the first matmuls.
  - Input DMAs split across the two HWDGE queues (sync carries the K/Q
    operands, scalar everything else) so the first matmuls start right
    after the NEFF preamble.
  - Projections accumulate into [128,2048] PSUM tiles (4 banks, one
    8-bank pool of 2) and drain with FD=2048 ACT ops, so ACT no longer
    paces the PE during phase 1 (v2 lost ~146ns per 2-matmul group).
  - co and PV produce TRANSPOSED tiles (partitions = output feature):
    bc+relu fuse into the co ACT drain as a per-partition bias, bv
    enters the PV PSUM group via a ones-row matmul (rows sum to 1 after
    L1-normalize, so +bv there lands as +bv/sqrt(D) in the output), and
    the epilogue stays one fused DVE op out^T = pv^T/sqrt(D) + co^T.
    The kernel emits out^T [D, nq]; the host transposes for free.
  - Scores run e4m3 DoubleRow into [128,1024] PSUM tiles; exp outputs
    bf16 at FD=1024.  Row sums come from an all-ones [128,128]
    stationary matmul so they land in PSUM already broadcast across
    partitions; one DVE reciprocal yields bf16 1/rowsum, and the
    normalize multiplies run in the DVE 2x bf16 mode, emitting e4m3
    probabilities in [0,1] for the PV DoubleRow matmul.
"""

import sys
from contextlib import ExitStack

for _p in ("/opt/trn_rl_repo", "/root/.axon_site/_ro/trn_rl_repo"):
    if _p not in sys.path:
        sys.path.insert(0, _p)

import ml_dtypes
import numpy as np

import concourse.bacc as bacc
import concourse.bass as bass
import concourse.mybir as mybir
import concourse.tile as tile

P = 128
D = 512
DC = D // P   # contraction chunks of 128
CW = 1024     # DMA chunk width (columns)
INV_SQRT_D = float(1.0 / np.sqrt(D))
SQRT_D = float(np.sqrt(D))
EXP_SHIFT = -40.0

F32 = mybir.dt.float32
BF16 = mybir.dt.bfloat16
FP8 = mybir.dt.float8e4
AF = mybir.ActivationFunctionType
ALU = mybir.AluOpType
DR = mybir.MatmulPerfMode.DoubleRow


def build_program(nq, nkv, reps=1):
    nc = bacc.Bacc("TRN2", target_bir_lowering=False, debug=False, num_devices=8)

    NCH = nkv // CW
    NCHh = nq // CW
    # all host-pre-arranged, partition-major, fat contiguous runs
    xA8 = nc.dram_tensor("xA8", [P, NCH * DC * CW], FP8, kind="ExternalInput").ap()
    xB8 = nc.dram_tensor("xB8", [P, NCH * DC * CW], FP8, kind="ExternalInput").ap()
    xAh = nc.dram_tensor("xAh", [P, NCHh * DC * CW], BF16, kind="ExternalInput").ap()
    xBh = nc.dram_tensor("xBh", [P, NCHh * DC * CW], BF16, kind="ExternalInput").ap()
    wq8 = nc.dram_tensor("wq8", [P, DC * D], FP8, kind="ExternalInput").ap()
    wk8 = nc.dram_tensor("wk8", [P, DC * D], FP8, kind="ExternalInput").ap()
    wv8 = nc.dram_tensor("wv8", [P, DC * D], FP8, kind="ExternalInput").ap()
    wcA = nc.dram_tensor("wcA", [P, DC * D], BF16, kind="ExternalInput").ap()
    wcB = nc.dram_tensor("wcB", [P, DC * D], BF16, kind="ExternalInput").ap()
    # bq | bk | bc columns packed into one tensor: one tiny DMA, loaded
    # FIRST -- a late bias blocks the projection ACT drains, which holds
    # PSUM and stalls the PE for ~10us
    b3_pc = nc.dram_tensor("b3_pc", [P, 3 * DC], F32, kind="ExternalInput").ap()
    bv_row = nc.dram_tensor("bv_row", [1, D], BF16, kind="ExternalInput").ap()
    out = nc.dram_tensor("out", [D, nq], F32, kind="ExternalOutput").ap()

    with tile.TileContext(nc) as tc:
        for rep in range(reps):
            _emit_body(
                nc, tc, xA8, xB8, xAh, xBh, wq8, wk8, wv8, wcA, wcB,
                b3_pc, bv_row, out, nq, nkv, rep,
            )

    nc.compile()
    return nc


def _emit_body(
    nc, tc, xA8, xB8, xAh, xBh, wq8, wk8, wv8, wcA, wcB,
    b3_pc, bv_row, out, nq, nkv, rep,
):
    NBQ = nq // P   # query row blocks (16)
    MCK = nkv // P  # key row chunks (32)
    NCH = nkv // CW
    NCHh = nq // CW

    with ExitStack() as st:
        persist = st.enter_context(tc.tile_pool(name="persist", bufs=1))

        # ---------------- phase 1: projections (all local) ----------------
        with ExitStack() as ph1:
            xp = ph1.enter_context(tc.tile_pool(name="xp", bufs=1))
            ps1 = ph1.enter_context(tc.tile_pool(name="ps1", bufs=3, space="PSUM"))
            wmp = ph1.enter_context(tc.tile_pool(name="wmp", bufs=1, space="PSUM"))

            # x8/xh chunks alternate between the two HWDGE queues (sync +
            # scalar): each queue executes its DMAs serially incl. a ~2us
            # completion latency, so one-queue delivery starved the first
            # matmuls for ~11us.
            def _chunk(dram, ci):
                return dram[:, ci * DC * CW : (ci + 1) * DC * CW].rearrange(
                    "p (c w) -> p c w", c=DC
                )

            wk_sb = persist.tile([P, DC, D], FP8, name="w_wk")
            nc.sync.dma_start(out=wk_sb, in_=wk8.rearrange("p (c o) -> p c o", c=DC))
            b3_sb = persist.tile([P, 3 * DC], F32, name="b3_sb")
            nc.sync.dma_start(out=b3_sb, in_=b3_pc)
            bq_sb = b3_sb[:, 0:DC]
            bk_sb = b3_sb[:, DC : 2 * DC]
            bc_sb = b3_sb[:, 2 * DC : 3 * DC]
            wq_sb = persist.tile([P, DC, D], FP8, name="w_wq")
            nc.scalar.dma_start(out=wq_sb, in_=wq8.rearrange("p (c o) -> p c o", c=DC))
            bv_sb = persist.tile([1, D], BF16, name="bv_sb")
            nc.scalar.dma_start(out=bv_sb, in_=bv_row)
            xB8_sb = xp.tile([P, NCH, DC, CW], FP8, name="xB8_sb")
            xA8_sb = xp.tile([P, NCH, DC, CW], FP8, name="xA8_sb")
            for ci in range(NCH):
                eng = nc.sync if ci % 2 == 0 else nc.scalar
                eng.dma_start(out=xB8_sb[:, ci], in_=_chunk(xB8, ci))
            wv_sb = persist.tile([P, DC, D], FP8, name="w_wv")
            nc.scalar.dma_start(out=wv_sb, in_=wv8.rearrange("p (c o) -> p c o", c=DC))
            for ci in range(NCH):
                eng = nc.sync if ci % 2 == 1 else nc.scalar
                eng.dma_start(out=xA8_sb[:, ci], in_=_chunk(xA8, ci))

            wcA_sb = persist.tile([P, DC, D], BF16, name="w_wcA")
            nc.scalar.dma_start(out=wcA_sb, in_=wcA.rearrange("p (c o) -> p c o", c=DC))
            wcB_sb = persist.tile([P, DC, D], BF16, name="w_wcB")
            nc.sync.dma_start(out=wcB_sb, in_=wcB.rearrange("p (c o) -> p c o", c=DC))
            xAh_sb = xp.tile([P, NCHh, DC, CW], BF16, name="xAh_sb")
            xBh_sb = xp.tile([P, NCHh, DC, CW], BF16, name="xBh_sb")
            for ci in range(NCHh):
                nc.sync.dma_start(out=xAh_sb[:, ci], in_=_chunk(xAh, ci))
                nc.scalar.dma_start(out=xBh_sb[:, ci], in_=_chunk(xBh, ci))

            ones_row = persist.tile([1, 512], BF16, name="ones_row")
            nc.vector.memset(ones_row, 1.0)
            ones_mat = persist.tile([P, P], BF16, name="ones_mat")
            nc.vector.memset(ones_mat, 1.0)
            shift_sb = persist.tile([P, 1], F32, name="shift_sb")
            nc.vector.memset(shift_sb, EXP_SHIFT)

            # warm-up while the first DMAs land: junk matmuls hold the PE
            # HAM clock-gate at 2.4 GHz for the real work, and a 1-column
            # exp preloads the ACT table set (~2.7us otherwise paid right
            # before the first score tile).
            warm_act = persist.tile([P, 1], F32, name="warm_act")
            nc.scalar.activation(out=warm_act, in_=shift_sb, func=AF.Exp)
            warm_ps = wmp.tile([P, 512], F32, name="warm_ps")
            for _wi in range(8):
                nc.tensor.matmul(
                    warm_ps,
                    lhsT=ones_row[:, :P],
                    rhs=ones_row,
                    start=True,
                    stop=True,
                )

            qT_sb = persist.tile([P, DC, nq], FP8, name="qT_sb")
            kT_sb = persist.tile([P, DC, nkv], FP8, name="kT_sb")
            v_sb = persist.tile([P, MCK, D], FP8, name="v_sb")
            coT_sb = persist.tile([P, DC, nq], BF16, name="coT_sb")

            def _junk(n):
                # junk matmuls into the dedicated warm bank: keep the PE
                # HAM window busy across DMA-delivery hiccups -- a >3.4us
                # idle re-throttles the clock to 1.2 GHz and every matmul
                # after runs at half rate for the next ~3.4us
                for _wi in range(n):
                    nc.tensor.matmul(
                        warm_ps,
                        lhsT=ones_row[:, :P],
                        rhs=ones_row,
                        start=True,
                        stop=True,
                    )

            # K over all keys (fp8 DoubleRow), then Q over own rows.  Every
            # third PSUM drain goes to the DVE (tensor_scalar_add with the
            # per-partition bias AP): ACT alone drains at 997ns/tile vs
            # the PE's 864ns fill, costing ~133ns of PE stall per tile.
            drain_i = 0
            for s1 in range(0, nkv, 1024):
                for ob in range(DC):
                    ps = ps1.tile([P, 1024], F32, name="ps1t", tag="ps1")
                    for q4 in range(2):
                        s0 = s1 + q4 * 512
                        ci, off = s0 // CW, s0 % CW
                        for c2 in range(DC // 2):
                            nc.tensor.matmul(
                                ps[:, q4 * 512 : (q4 + 1) * 512],
                                lhsT=wk_sb[:, 2 * c2 : 2 * c2 + 2, ob * P : (ob + 1) * P],
                                rhs=xB8_sb[:, ci, 2 * c2 : 2 * c2 + 2, off : off + 512],
                                start=(c2 == 0),
                                stop=(c2 == DC // 2 - 1),
                                perf_mode=DR,
                            )
                    if drain_i % 3 == 2:
                        nc.vector.tensor_scalar_add(
                            out=kT_sb[:, ob, s1 : s1 + 1024],
                            in0=ps,
                            scalar1=bk_sb[:, ob : ob + 1],
                        )
                    else:
                        nc.scalar.activation(
                            out=kT_sb[:, ob, s1 : s1 + 1024],
                            in_=ps,
                            func=AF.Identity,
                            bias=bk_sb[:, ob : ob + 1],
                            scale=1.0,
                        )
                    drain_i += 1
                    if s1 <= 1024:
                        _junk(2)
            for s1 in range(0, nq, 1024):
                for ob in range(DC):
                    ps = ps1.tile([P, 1024], F32, name="ps1t", tag="ps1")
                    for q4 in range(2):
                        s0 = s1 + q4 * 512
                        ci, off = s0 // CW, s0 % CW
                        for c2 in range(DC // 2):
                            nc.tensor.matmul(
                                ps[:, q4 * 512 : (q4 + 1) * 512],
                                lhsT=wq_sb[:, 2 * c2 : 2 * c2 + 2, ob * P : (ob + 1) * P],
                                rhs=xA8_sb[:, ci, 2 * c2 : 2 * c2 + 2, off : off + 512],
                                start=(c2 == 0),
                                stop=(c2 == DC // 2 - 1),
                                perf_mode=DR,
                            )
                    if drain_i % 3 == 2:
                        nc.vector.tensor_scalar_add(
                            out=qT_sb[:, ob, s1 : s1 + 1024],
                            in0=ps,
                            scalar1=bq_sb[:, ob : ob + 1],
                        )
                    else:
                        nc.scalar.activation(
                            out=qT_sb[:, ob, s1 : s1 + 1024],
                            in_=ps,
                            func=AF.Identity,
                            bias=bq_sb[:, ob : ob + 1],
                            scale=1.0,
                        )
                    drain_i += 1

            # V over all key rows of the A view (fp8 DoubleRow), e4m3 out;
            # emitted before co so the PE has work while co's bf16 x
            # halves are still streaming in
            for mg in range(0, MCK, 2):
                ps = ps1.tile([P, 1024], F32, name="ps1t", tag="ps1")
                for mi in range(2):
                    m = mg + mi
                    ci, off = (m * P) // CW, (m * P) % CW
                    for c2 in range(DC // 2):
                        nc.tensor.matmul(
                            ps[:, mi * 512 : (mi + 1) * 512],
                            lhsT=xA8_sb[:, ci, 2 * c2 : 2 * c2 + 2, off : off + P],
                            rhs=wv_sb[:, 2 * c2 : 2 * c2 + 2, :],
                            start=(c2 == 0),
                            stop=(c2 == DC // 2 - 1),
                            perf_mode=DR,
                        )
                if drain_i % 3 == 2:
                    nc.vector.tensor_copy(v_sb[:, mg : mg + 2, :], ps)
                else:
                    nc.scalar.activation(
                        out=v_sb[:, mg : mg + 2, :], in_=ps, func=AF.Copy
                    )
                drain_i += 1

            # co^T = relu(WcA^T xA + WcB^T xB + bc): transposed (feature on
            # partitions) so bc+relu fuse into the ACT drain per-partition
            for ob in range(DC):
                for s1 in range(0, nq, 1024):
                    ps = ps1.tile([P, 1024], F32, name="ps1t", tag="ps1")
                    for q4 in range(2):
                        s0 = s1 + q4 * 512
                        ci, off = s0 // CW, s0 % CW
                        first = True
                        for w_sb, xh_sb in ((wcA_sb, xAh_sb), (wcB_sb, xBh_sb)):
                            for c in range(DC):
                                last = w_sb is wcB_sb and c == DC - 1
                                nc.tensor.matmul(
                                    ps[:, q4 * 512 : (q4 + 1) * 512],
                                    lhsT=w_sb[:, c, ob * P : (ob + 1) * P],
                                    rhs=xh_sb[:, ci, c, off : off + 512],
                                    start=first,
                                    stop=last,
                                )
                                first = False
                    nc.scalar.activation(
                        out=coT_sb[:, ob, s1 : s1 + 1024],
                        in_=ps,
                        func=AF.Relu,
                        bias=bc_sb[:, ob : ob + 1],
                        scale=1.0,
                    )

        # ---------------- phase 2: attention (S^T layout) ----------------
        at_pool = st.enter_context(tc.tile_pool(name="at_pool", bufs=2))
        a8_pool = st.enter_context(tc.tile_pool(name="a8_pool", bufs=2))
        r_pool = st.enter_context(tc.tile_pool(name="r_pool", bufs=2))
        o_pool = st.enter_context(tc.tile_pool(name="o_pool", bufs=4))
        sps_pool = st.enter_context(tc.tile_pool(name="sps", bufs=2, space="PSUM"))
        sum_pool = st.enter_context(tc.tile_pool(name="sump", bufs=1, space="PSUM"))
        pv_pool = st.enter_context(tc.tile_pool(name="pv", bufs=3, space="PSUM"))

        for s0 in range(0, nq, 512):
            at_sb = at_pool.tile([P, MCK, 512], BF16, name="at_sb", tag="at")
            at8 = a8_pool.tile([P, MCK, 512], FP8, name="at8", tag="at8")
            # rowsum, broadcast across partitions by the all-ones lhsT
            ssum = sum_pool.tile([P, 512], F32, name="ssum", tag="ssum")
            for mbp in range(MCK // 2):
                sps = sps_pool.tile([P, 1024], F32, name="sps", tag="sps")
                for mi in range(2):
                    mb = 2 * mbp + mi
                    for c2 in range(DC // 2):
                        nc.tensor.matmul(
                            sps[:, mi * 512 : (mi + 1) * 512],
                            lhsT=kT_sb[:, 2 * c2 : 2 * c2 + 2, mb * P : (mb + 1) * P],
                            rhs=qT_sb[:, 2 * c2 : 2 * c2 + 2, s0 : s0 + 512],
                            start=(c2 == 0),
                            stop=(c2 == DC // 2 - 1),
                            perf_mode=DR,
                        )
                nc.scalar.activation(
                    out=at_sb[:, 2 * mbp : 2 * mbp + 2, :],
                    in_=sps,
                    func=AF.Exp,
                    bias=shift_sb,
                    scale=1.0,
                )
                for mi in range(2):
                    mb = 2 * mbp + mi
                    nc.tensor.matmul(
                        ssum,
                        lhsT=ones_mat,
                        rhs=at_sb[:, mb, :],
                        start=(mb == 0),
                        stop=(mb == MCK - 1),
                    )
            # 1/rowsum (already broadcast).  The exact DVE reciprocal is an
            # 8-cycle/element iterative divide (~3.4us, head-of-line blocks
            # the PE); approx_fast is ~51 ULP -- far below the bf16
            # rounding of rs_bc that the normalize already accepts.
            # rowsum >= exp(-40+max_score) stays well above denormals.
            rs_f32 = r_pool.tile([P, 512], F32, name="rs_f32", tag="rsf")
            nc.vector.reciprocal_approx_fast(out=rs_f32, in_=ssum)
            rs_bc = r_pool.tile([P, 512], BF16, name="rs_bc", tag="rsb")
            with nc.allow_low_precision(reason="bf16 1/rowsum: 0.4% on a term ~5% of |out|"):
                nc.vector.tensor_copy(rs_bc, rs_f32)
            for mb in range(MCK):
                nc.vector.tensor_mul(at8[:, mb, :], at_sb[:, mb, :], rs_bc)

            # PV^T: partitions = output feature; bv enters the PSUM group
            # via ones x bv (rows sum to 1 -> lands as +bv/sqrt(D)).
            # j=0..2 interleave across the key loop so the PE consumes the
            # normalize multiplies at the DVE's production rate instead of
            # stalling ~400ns per at8 pair; j=3 then runs dense.
            def _pv_group(js):
                pvs = []
                for j in js:
                    pv = pv_pool.tile([P, 512], F32, name="pv", tag="pv")
                    nc.tensor.matmul(
                        pv,
                        lhsT=bv_sb[:, j * P : (j + 1) * P],
                        rhs=ones_row,
                        start=True,
                        stop=False,
                    )
                    pvs.append(pv)
                for i2 in range(MCK // 2):
                    for j, pv in zip(js, pvs):
                        nc.tensor.matmul(
                            pv,
                            lhsT=v_sb[:, 2 * i2 : 2 * i2 + 2, j * P : (j + 1) * P],
                            rhs=at8[:, 2 * i2 : 2 * i2 + 2, :],
                            start=False,
                            stop=(i2 == MCK // 2 - 1),
                            perf_mode=DR,
                        )
                for j, pv in zip(js, pvs):
                    outt = o_pool.tile([P, 512], F32, name="outt", tag="outt")
                    nc.vector.scalar_tensor_tensor(
                        out=outt,
                        in0=pv,
                        scalar=INV_SQRT_D,
                        in1=coT_sb[:, j, s0 : s0 + 512],
                        op0=ALU.mult,
                        op1=ALU.add,
                    )
                    nc.sync.dma_start(
                        out=out[j * P : (j + 1) * P, s0 : s0 + 512], in_=outt
                    )

            _pv_group([0, 1, 2])
            _pv_group([3])


_PROG_CACHE = {}


def _get_program(nq, nkv):
    key = (nq, nkv)
    if key not in _PROG_CACHE:
        _PROG_CACHE[key] = build_program(nq, nkv)
    return _PROG_CACHE[key]


def _pc_chunks(xT, dt):
    """[D, N] feature-major -> [P, NCH*DC*CW] partition-major fat runs."""
    Dd, N = xT.shape
    nch = N // CW
    return np.ascontiguousarray(
        xT.reshape(DC, P, nch, CW).transpose(1, 2, 0, 3).reshape(P, -1)
    ).astype(dt)


def _pc_weight(WT, dt):
    """[D, D] (in, out) -> [P, DC*D] partition-major."""
    return np.ascontiguousarray(
        WT.reshape(DC, P, D).transpose(1, 0, 2).reshape(P, -1)
    ).astype(dt)


def make_in_maps(x_f, x_s, Wq, bq, Wk, bk, Wv, bv, Wc, bc):
    """Per-core SPMD input dicts + (direction, batch, half) layout."""
    x_f = np.asarray(x_f, np.float32)
    x_s = np.asarray(x_s, np.float32)
    B, N, _ = x_f.shape
    nq = N // 2
    bf = ml_dtypes.bfloat16
    e4 = ml_dtypes.float8_e4m3
    Wq8 = _pc_weight(np.asarray(Wq, np.float32).T, e4)
    Wk8 = _pc_weight(np.asarray(Wk, np.float32).T, e4)
    Wv8 = _pc_weight(np.asarray(Wv, np.float32).T, e4)
    Wc = np.asarray(Wc, np.float32)
    WcfT = _pc_weight(np.ascontiguousarray(Wc[:, :D].T), bf)
    WcsT = _pc_weight(np.ascontiguousarray(Wc[:, D:].T), bf)
    bq32, bk32, bv32, bc32 = (
        np.asarray(b, np.float32) for b in (bq, bk, bv, bc)
    )
    b3_pc = np.ascontiguousarray(
        np.concatenate(
            [b.reshape(DC, P).T for b in (bq32, bk32, bc32)], axis=1
        )
    )
    # rows sum to 1 after L1-normalize, so +bv in the PV PSUM lands as
    # +bv/sqrt(D) in the output after the epilogue's 1/sqrt(D) scale --
    # exactly the reference's V-projection bias term
    bv_row = np.ascontiguousarray(bv32[None, :]).astype(bf)
    xT_f = [np.ascontiguousarray(x_f[b].T) for b in range(B)]
    xT_s = [np.ascontiguousarray(x_s[b].T) for b in range(B)]
    in_maps, layout = [], []
    for d in range(2):
        for b in range(B):
            for h in range(2):
                xq = xT_f[b] if d == 0 else xT_s[b]
                xk = xT_s[b] if d == 0 else xT_f[b]
                if h == 1:
                    idx = np.r_[nq:N, 0:nq]
                    xq, xk = xq[:, idx], xk[:, idx]
                in_maps.append(
                    {
                        "xA8": _pc_chunks(xq, e4),
                        "xB8": _pc_chunks(xk, e4),
                        "xAh": _pc_chunks(xq[:, :nq], bf),
                        "xBh": _pc_chunks(xk[:, :nq], bf),
                        "wq8": Wq8,
                        "wk8": Wk8,
                        "wv8": Wv8,
                        "wcA": WcfT if d == 0 else WcsT,
                        "wcB": WcsT if d == 0 else WcfT,
                        "b3_pc": b3_pc,
                        "bv_row": bv_row,
                    }
                )
                layout.append((d, b, h))
    return in_maps, layout


def kernel(x_f, x_s, Wq, bq, Wk, bk, Wv, bv, Wc, bc):
    x_f = np.asarray(x_f, np.float32)
    B, N, _ = x_f.shape
    nq = N // 2
    nc = _get_program(nq, N)
    in_maps, layout = make_in_maps(x_f, x_s, Wq, bq, Wk, bk, Wv, bv, Wc, bc)

    from concourse.bass_utils import run_bass_kernel_spmd

    res = run_bass_kernel_spmd(nc, in_maps, list(range(len(in_maps))))
    out_f = np.empty((B, N, D), np.float32)
    out_s = np.empty((B, N, D), np.float32)
    for (d, b, h), r in zip(layout, res.results):
        tgt = out_f if d == 0 else out_s
        tgt[b, h * nq : (h + 1) * nq] = r["out"].T
    return out_f, out_s
